# revision 48
# baseline (speedup 1.0000x reference)
"""Trainium2 Bass kernel for nn_DA_conv1D (dynamic depthwise conv1d + 1x1 conv
+ channel-attention gate), data-parallel over batch on 8 NeuronCores.

Shapes (hardcoded): x0 [32, 64, 16384] f32, x1 [32, 64] f32.
Each core handles 4 samples, organized as 2 "pairs" of 2 samples so the
128 SBUF partitions hold (2 samples x 64 channels).

The length axis is pre-chunked on host into per-(pair, chunk) DRAM tensors
that already include the 1-column halo on each side (edge halos hold the
bias/gate compensation value d so padded taps cancel exactly).  Each chunk
is one contiguous DRAM block -> one large DMA descriptor per queue instead
of 128 x 2KB strided rows.  Chunk sizes taper at the stream edges (pair 0
starts small, pair 1 ends small) to shrink pipeline fill/drain.

Per pair the chunk is processed in <=1024-wide groups (2 PSUM banks),
software-pipelined one group deep:
  S1  ps1 = sum_j diag(kern_j) @ x_shift_j   (PE, bf16 matmuls, tap-major)
  S2  lr  = lrelu(ps1)                       (ACT Prelu, PSUM->SBUF, bf16)
  S3  ps2 = blockdiag(conv_w) @ lr           (PE; issued after the NEXT
                                              group's S1 so the PE never
                                              waits on this group's ACT)
  S4  out = x0 * att + ps2                   (DVE stt, writes bf16)

Output is stored in bf16 (half the store traffic; tolerance is 2e-2) and
upcast to fp32 on host.  The tiny dynamic-weight math (h = lrelu(x1 W1^T),
kern = h W2^T, SE gate att = sigmoid(lrelu(x1 ca_w1^T) ca_w2^T)) is
computed on host in fp32 and shipped as per-core diagonal/gate tensors.
"""

import os
import sys

for _p in ("/opt/trn_rl_repo", "/root/.axon_site/_ro/trn_rl_repo"):
    if os.path.isdir(_p) and _p not in sys.path:
        sys.path.append(_p)

import ml_dtypes
import numpy as np

import concourse.bacc as bacc
import concourse.tile as tile
from concourse import mybir
from concourse.bass_utils import run_bass_kernel_spmd

B, C, L, K = 32, 64, 16384, 3
N_CORES = 8
SAMPLES_PER_CORE = B // N_CORES          # 4
PAIRS = SAMPLES_PER_CORE // 2            # 2
P = 128                                  # SBUF partitions = 2 samples x 64 ch
CHUNK = 2048                             # max chunk (SBUF tile size)
# tapered, asymmetric: pair 0 ramps up (short fill), pair 1 ramps down
# (short drain); the interior runs at the full 2048 chunk size
CHUNK_SIZES = [
    [256, 768, 1024] + [2048] * 7,       # pair 0
    [2048] * 7 + [1536, 256, 256],       # pair 1
]
GTILE = 1024                             # max ACT/DVE group width (2 banks)
NTILE = 512                              # matmul moving width (PSUM bank)
F32 = mybir.dt.float32
BF16 = mybir.dt.bfloat16
BF16_NP = ml_dtypes.bfloat16

TRACE = False          # test harness flips this to profile
USE_LRELU = True       # HW Prelu activation (CoreSim lacks it; see simcheck)
LAST_RESULT = None     # BassKernelResults of the most recent run

_COMPILED = {}         # (use_lrelu,) -> compiled Bacc program


def _groups(csz):
    """Split a chunk into <=GTILE-wide groups."""
    out = []
    u = 0
    while u < csz:
        g = min(GTILE, csz - u)
        out.append((u, g))
        u += g
    return out


def _build_program(use_lrelu):
    nc = bacc.Bacc("TRN2", target_bir_lowering=False, debug=False,
                   num_devices=N_CORES)

    # per-(pair, chunk) input blocks, halo included: col i = x0[lo - 1 + i]
    xin = [[nc.dram_tensor(f"xin_{p}_{c}", [P, csz + 2], BF16,
                           kind="ExternalInput").ap()
            for c, csz in enumerate(CHUNK_SIZES[p])] for p in range(PAIRS)]
    xout = [[nc.dram_tensor(f"out_{p}_{c}", [P, csz], BF16,
                            kind="ExternalOutput").ap()
             for c, csz in enumerate(CHUNK_SIZES[p])] for p in range(PAIRS)]
    # diag kernels pre-flattened per partition: [(pair, tap) -> 128 cols]
    diags = nc.dram_tensor("diags", [P, PAIRS * K * P], BF16,
                           kind="ExternalInput").ap()
    # scal[:, 0:PAIRS] = att per pair; scal[:, PAIRS:2*PAIRS] = prelu bias
    # (-sum_j kern_j * d, the depthwise compensation for the host-side
    #  x0 + d shift that folds conv_b into the residual term);
    # scal[:, 2*PAIRS + p*K + j] = kern tap j of pair p (bf16-rounded, f32)
    scal = nc.dram_tensor("scal", [P, 2 * PAIRS + PAIRS * K], F32,
                          kind="ExternalInput").ap()
    wblk = nc.dram_tensor("wblk", [P, P], BF16, kind="ExternalInput").ap()

    mult = mybir.AluOpType.mult
    add = mybir.AluOpType.add
    Relu = mybir.ActivationFunctionType.Relu
    Prelu = mybir.ActivationFunctionType.Prelu
    Ident = mybir.ActivationFunctionType.Identity

    with tile.TileContext(nc) as tc:
        with (
            tc.tile_pool(name="consts", bufs=1) as consts,
            tc.tile_pool(name="xbf", bufs=8) as xbf_pool,
            tc.tile_pool(name="lr", bufs=4) as lr_pool,
            tc.tile_pool(name="dvetmp", bufs=2) as dve_pool,
            tc.tile_pool(name="outc", bufs=4) as out_pool,
            tc.tile_pool(name="ps1", bufs=2, space="PSUM") as ps1_pool,
            tc.tile_pool(name="ps2", bufs=2, space="PSUM") as ps2_pool,
        ):
            # diag_t and the first chunk gate the first depthwise matmul:
            # issue them on DIFFERENT queues (sync / scalar) so the two
            # transfers overlap; wblk/scal follow on scalar (needed later)
            diag_t = consts.tile([P, PAIRS * K * P], BF16)
            nc.sync.dma_start(diag_t[:], diags[:])
            sz0 = CHUNK_SIZES[0][0]
            first_xbf = xbf_pool.tile([P, CHUNK + 2], BF16, tag="xbf")
            nc.sync.dma_start(first_xbf[:, 0:sz0 + 2], xin[0][0])

            wblk_t = consts.tile([P, P], BF16)
            nc.scalar.dma_start(wblk_t[:], wblk[:])
            scal_t = consts.tile([P, 2 * PAIRS + PAIRS * K], F32)
            nc.scalar.dma_start(scal_t[:], scal[:])
            att = [scal_t[:, p:p + 1] for p in range(PAIRS)]
            pb = [scal_t[:, PAIRS + p:PAIRS + p + 1] for p in range(PAIRS)]
            kap = [[scal_t[:, 2 * PAIRS + p * K + j:2 * PAIRS + p * K + j + 1]
                    for j in range(K)] for p in range(PAIRS)]

            # software pipeline for the lrelu + 1x1 + combine + store
            # stages: normal groups finish one iteration later; DVE-offload
            # groups finish three iterations later (their depthwise chain
            # is ~2us of serial DVE latency — deferring keeps the in-order
            # ACT/PE streams from stalling behind it).  Output stores ride
            # the sync queue (no gpsimd SWDGE): each store is issued right
            # AFTER the next chunk's input DMA so its wait (on that chunk's
            # combine) never delays input prefetch.
            pending = []   # [lag, is_conv, payload]
            pending_store = []

            def finish(prev):
                p_, lr_, xbf_, outc_, u_, gsz_, dma_ = prev
                ps2 = ps2_pool.tile([P, GTILE], F32)
                for h in range((gsz_ + NTILE - 1) // NTILE):
                    nc.tensor.matmul(
                        ps2[:, h * NTILE:h * NTILE + min(NTILE, gsz_ - h * NTILE)],
                        wblk_t[:],
                        lr_[:, h * NTILE:h * NTILE + min(NTILE, gsz_ - h * NTILE)],
                        start=True, stop=True)
                nc.vector.scalar_tensor_tensor(
                    outc_[:, u_:u_ + gsz_], xbf_[:, u_ + 1:u_ + 1 + gsz_],
                    att[p_], ps2[:, :gsz_], op0=mult, op1=add)
                if dma_ is not None:
                    pending_store.append(dma_)

            def step():
                # issue finishes that are due (normal groups first so the
                # PE never waits on a just-issued deferred lrelu)
                for e in pending:
                    e[0] -= 1
                for want_conv in (False, True):
                    for e in [e for e in pending
                              if e[0] <= 0 and e[1] == want_conv]:
                        pl = e[2]
                        if e[1]:
                            p_, t3_, xbf_, outc_, u_, gsz_, dma_ = pl
                            lr = lr_pool.tile([P, GTILE], BF16)
                            nc.scalar.activation(lr[:], t3_[:], Prelu,
                                                 bias=pb[p_], alpha=0.1)
                            pl = (p_, lr, xbf_, outc_, u_, gsz_, dma_)
                        finish(pl)
                        pending.remove(e)

            # these (full-1024, mid-stream) groups compute the depthwise on
            # DVE+ACT instead of the PE, balancing engine load: the PE is
            # the bottleneck (~57us busy) while DVE/ACT have headroom
            convert = {4, 10, 16, 22, 28} if use_lrelu else set()
            gidx = 0

            for p in range(PAIRS):
                for c, csz in enumerate(CHUNK_SIZES[p]):
                    # xbf[:, i] = x0[lo + i - 1]  (halo pre-packed on host)
                    if p == 0 and c == 0:
                        xbf = first_xbf
                    else:
                        xbf = xbf_pool.tile([P, CHUNK + 2], BF16, tag="xbf")
                        nc.sync.dma_start(xbf[:, 0:csz + 2], xin[p][c])
                        while pending_store:
                            nc.sync.dma_start(*pending_store.pop(0))

                    outc = out_pool.tile([P, CHUNK], BF16, tag="outc")
                    groups = _groups(csz)
                    for gi, (u, gsz) in enumerate(groups):
                        if gidx in convert:
                            # dw on DVE+ACT: t3 = sum_j kern_j * x_shift_j
                            assert gsz == GTILE
                            t1 = dve_pool.tile([P, GTILE], BF16, tag="t1")
                            nc.vector.tensor_scalar_mul(
                                t1[:], xbf[:, u:u + gsz], kap[p][0])
                            todd = dve_pool.tile([P, GTILE], BF16,
                                                 tag="todd")
                            nc.scalar.activation(
                                todd[:], xbf[:, u + 1:u + 1 + gsz], Ident,
                                scale=kap[p][1])
                            step()
                            t2 = dve_pool.tile([P, GTILE], BF16, tag="t2")
                            nc.vector.scalar_tensor_tensor(
                                t2[:], xbf[:, u + 2:u + 2 + gsz], kap[p][2],
                                t1[:], op0=mult, op1=add)
                            t3 = dve_pool.tile([P, GTILE], BF16, tag="t3")
                            nc.vector.tensor_add(t3[:], t2[:], todd[:])
                            dma = None
                            if gi == len(groups) - 1:
                                dma = (xout[p][c], outc[:, :csz])
                            pending.append(
                                [3, True, (p, t3, xbf, outc, u, gsz, dma)])
                            gidx += 1
                            continue
                        # S1: depthwise matmuls, tap-major (lhsT reuse)
                        ps1 = ps1_pool.tile([P, GTILE], F32)
                        for j in range(K):
                            for h in range((gsz + NTILE - 1) // NTILE):
                                n = min(NTILE, gsz - h * NTILE)
                                nc.tensor.matmul(
                                    ps1[:, h * NTILE:h * NTILE + n],
                                    diag_t[:, (p * K + j) * P:
                                           (p * K + j + 1) * P],
                                    xbf[:, u + h * NTILE + j:
                                        u + h * NTILE + j + n],
                                    start=(j == 0), stop=(j == K - 1),
                                )
                        # S3/S4/store of due groups (PE issues their 1x1
                        # after this group's depthwise, so it never stalls
                        # on the freshly-issued ACT below)
                        step()
                        # S2: lrelu
                        lr = lr_pool.tile([P, GTILE], BF16)
                        if use_lrelu:
                            nc.scalar.activation(lr[:, :gsz], ps1[:, :gsz],
                                                 Prelu, bias=pb[p], alpha=0.1)
                        else:
                            tt = lr_pool.tile([P, GTILE], F32, tag="tt")
                            nc.scalar.activation(tt[:, :gsz], ps1[:, :gsz],
                                                 Ident, bias=pb[p])
                            r9 = lr_pool.tile([P, GTILE], F32, tag="r9")
                            nc.scalar.activation(r9[:, :gsz], tt[:, :gsz],
                                                 Relu, scale=0.9)
                            nc.vector.scalar_tensor_tensor(
                                lr[:, :gsz], tt[:, :gsz], 0.1, r9[:, :gsz],
                                op0=mult, op1=add)
                        dma = None
                        if gi == len(groups) - 1:
                            dma = (xout[p][c], outc[:, :csz])
                        pending.append(
                            [2, False, (p, lr, xbf, outc, u, gsz, dma)])
                        gidx += 1
            while pending:
                step()
            while pending_store:
                nc.sync.dma_start(*pending_store.pop(0))

    nc.compile()
    return nc


def _lrelu(x):
    return np.where(x >= 0, x, np.float32(0.1) * x)


def kernel(x0, x1, W1, W2, conv_w, conv_b, ca_w1, ca_w2):
    global LAST_RESULT
    x0 = np.ascontiguousarray(np.asarray(x0, dtype=np.float32))
    x1 = np.asarray(x1, dtype=np.float32)
    W1 = np.asarray(W1, dtype=np.float32)
    W2 = np.asarray(W2, dtype=np.float32)
    conv_w = np.asarray(conv_w, dtype=np.float32)
    conv_b = np.asarray(conv_b, dtype=np.float32)
    ca_w1 = np.asarray(ca_w1, dtype=np.float32)
    ca_w2 = np.asarray(ca_w2, dtype=np.float32)

    # dynamic depthwise kernels + SE gate (tiny, fp32 host math)
    h = _lrelu(x1 @ W1.T)                                   # [B, 64]
    kern = (h @ W2.T).reshape(B, C, K)                      # [B, C, K]
    att = 1.0 / (1.0 + np.exp(-(_lrelu(x1 @ ca_w1.T) @ ca_w2.T)))
    att = att.astype(np.float32)                            # [B, C]

    # block-diagonal 1x1-conv weight as lhsT: lhsT[k, m] = W[m, k]
    wblk_np = np.zeros((P, P), np.float32)
    wblk_np[:C, :C] = conv_w.T
    wblk_np[C:, C:] = conv_w.T
    wblk_np = wblk_np.astype(BF16_NP)

    key = (USE_LRELU,)
    if key not in _COMPILED:
        _COMPILED[key] = _build_program(USE_LRELU)
    nc = _COMPILED[key]

    biasP = np.tile(conv_b, 2).astype(np.float32)            # [P]
    in_maps = []
    for core in range(N_CORES):
        s0 = core * SAMPLES_PER_CORE
        diags_np = np.zeros((P, PAIRS * K * P), np.float32)
        scal_np = np.empty((P, 2 * PAIRS + PAIRS * K), np.float32)
        in_map = {}
        for p in range(PAIRS):
            ka = kern[s0 + 2 * p]          # [C, K]
            kb = kern[s0 + 2 * p + 1]
            kern_bf = np.empty((P, K), np.float32)
            for j in range(K):
                s = (p * K + j) * P
                d = np.concatenate([ka[:, j], kb[:, j]])
                np.fill_diagonal(diags_np[:, s:s + P], d)
                kern_bf[:, j] = d.astype(BF16_NP).astype(np.float32)
            attp = np.concatenate([att[s0 + 2 * p], att[s0 + 2 * p + 1]])
            dp = biasP / attp                                 # [P]
            scal_np[:, p] = attp
            # depthwise compensation: -sum_j bf16(kern_j) * d
            scal_np[:, PAIRS + p] = -(kern_bf.sum(axis=1) * dp)
            for j in range(K):
                scal_np[:, 2 * PAIRS + p * K + j] = kern_bf[:, j]
            # shifted input for this pair, with halo columns; edge halo = d
            xp = x0[s0 + 2 * p:s0 + 2 * p + 2].reshape(P, L) + dp[:, None]
            xp8 = np.empty((P, L + 2), BF16_NP)
            xp8[:, 1:L + 1] = xp.astype(BF16_NP)
            dp8 = dp.astype(BF16_NP)
            xp8[:, 0] = dp8
            xp8[:, L + 1] = dp8
            lo = 0
            for c, csz in enumerate(CHUNK_SIZES[p]):
                in_map[f"xin_{p}_{c}"] = np.ascontiguousarray(
                    xp8[:, lo:lo + csz + 2])
                lo += csz
        in_map["diags"] = diags_np.astype(BF16_NP)
        in_map["scal"] = scal_np
        in_map["wblk"] = wblk_np
        in_maps.append(in_map)

    res = run_bass_kernel_spmd(nc, in_maps, list(range(N_CORES)), trace=TRACE)
    LAST_RESULT = res

    full = np.empty((B, C, L), np.float32)
    for core in range(N_CORES):
        s0 = core * SAMPLES_PER_CORE
        r = res.results[core]
        for p in range(PAIRS):
            cols = np.concatenate(
                [r[f"out_{p}_{c}"] for c in range(len(CHUNK_SIZES[p]))],
                axis=1)
            full[s0 + 2 * p:s0 + 2 * p + 2] = (
                cols.reshape(2, C, L).astype(np.float32))
    return full
